# revision 12
# baseline (speedup 1.0000x reference)
"""Trainium2 Bass kernel for a top-2 gated MoE layer (8 experts, H=1024, F=4096).

Strategy (expert parallelism across the 8 NeuronCores):
  - Host computes routing (top-2 over gate logits), the gate softmax
    weights, and the (O(T*H), compute-trivial) LayerNorm; it gathers each
    expert's normalized tokens into a padded, transposed activation block
    hdnT [H, C] bf16 (C = padded per-expert capacity).
  - Each core runs one expert's FFN only -- the 99.9%-of-FLOPs part:
    fc1 (bf16 matmul, fp32 PSUM) -> gelu(tanh)+b1 (ACT) -> fc2 (bf16
    matmul) -> y accumulation across the 4 F-blocks in SBUF (vector adds
    on the otherwise-idle DVE).  Weight tiles stay stationary across all
    C-chunks (one LDWEIGHTS per 3 matmuls -- a fresh lhsT every matmul
    costs ~46ns of unhidden LDWEIGHTS).
  - Host scatter-adds gate-weight-scaled per-expert outputs back into the
    full [B,S,H] tensor.

Self-contained: shapes are hardcoded from the problem spec.
"""

import numpy as np
import ml_dtypes
from contextlib import ExitStack

TOP_K = 2
LN_EPS = 1e-5
B, S, H, E, F = 2, 2048, 1024, 8, 4096
T = B * S
P = 128
KH = H // P          # 8 H-tiles (fc1 contraction / fc2 output)
MF1 = F // P         # 32 fc1 output tiles
FB = 1024            # F block size for weight DMA staging
NFB = F // FB        # 4 blocks
MB = FB // P         # 8 F-tiles per block
CW = 512             # chunk width (1 PSUM bank of fp32)

_BUILD_CACHE = {}


def _chunks(C):
    out = []
    off = 0
    while C - off > CW:
        out.append((off, CW))
        off += CW
    out.append((off, C - off))
    return out


def _build(C):
    """Build + compile the single-core Bass program (SPMD across 8 cores)."""
    if C in _BUILD_CACHE:
        return _BUILD_CACHE[C]

    import concourse.bass as bass  # noqa: F401
    import concourse.tile as tile
    import concourse.mybir as mybir
    from concourse import bacc

    bf = mybir.dt.bfloat16
    f32 = mybir.dt.float32
    AF = mybir.ActivationFunctionType

    nc = bacc.Bacc("TRN2", target_bir_lowering=False, debug=False, num_devices=8)

    d_hdn = nc.dram_tensor("hdnT", [H, C], bf, kind="ExternalInput")
    d_w1 = nc.dram_tensor("w1", [H, F], bf, kind="ExternalInput")
    d_w2 = nc.dram_tensor("w2", [F, H], bf, kind="ExternalInput")
    d_b1r = nc.dram_tensor("b1r", [P, MF1], f32, kind="ExternalInput")
    d_b2r = nc.dram_tensor("b2r", [P, KH], f32, kind="ExternalInput")
    d_y = nc.dram_tensor("ytT", [H, C], f32, kind="ExternalOutput")

    chunks = _chunks(C)
    NC = len(chunks)

    with tile.TileContext(nc) as tc, ExitStack() as ctx:
        const = ctx.enter_context(tc.tile_pool(name="const", bufs=1))
        xpool = ctx.enter_context(tc.tile_pool(name="x", bufs=1))
        w1pool = ctx.enter_context(tc.tile_pool(name="w1", bufs=2))
        w2pool = ctx.enter_context(tc.tile_pool(name="w2", bufs=2))
        apool = ctx.enter_context(tc.tile_pool(name="acts", bufs=2))
        ypool = ctx.enter_context(tc.tile_pool(name="y", bufs=1))
        ps1 = ctx.enter_context(tc.tile_pool(name="ps1", bufs=4, space="PSUM"))
        ps2 = ctx.enter_context(tc.tile_pool(name="ps2", bufs=4, space="PSUM"))

        w1ap = d_w1.ap().rearrange("(k p) f -> p k f", p=P)
        w2ap = d_w2.ap().rearrange("(k p) h -> p k h", p=P)

        def load_w1_block(fb, split=1):
            w1blk = w1pool.tile([P, KH, FB], bf, tag="w1", name=f"w1_{fb}")
            step = FB // split
            for i in range(split):
                fsl = slice(fb * FB + i * step, fb * FB + (i + 1) * step)
                nc.sync.dma_start(w1blk[:, :, i * step:(i + 1) * step],
                                  w1ap[:, :, fsl])
            return w1blk

        def load_w2_block(fb):
            w2blk = w2pool.tile([P, MB, H], bf, tag="w2", name=f"w2_{fb}")
            nc.sync.dma_start(w2blk[:], w2ap[:, fb * MB:(fb + 1) * MB, :])
            return w2blk

        # ---- DMAs ordered to match fc1(fb0)'s consumption: w1 block 0 in
        # per-m-tile pieces interleaved with per-k-tile hdn rows, so the
        # first PSUM group can start ~10us in; w2 deferred. ----
        hdn = xpool.tile([P, KH, C], bf, tag="x", name="hdn")
        d_xr = d_hdn.ap().rearrange("(k p) c -> p k c", p=P)
        w1blk0 = w1pool.tile([P, KH, FB], bf, tag="w1", name="w1_0")

        # k-granular pieces keep DMA descriptors at 2KB and land in the
        # order fb0's first PSUM group consumes them.
        b1_sb = const.tile([P, MF1], f32)
        b2_sb = const.tile([P, KH], f32)
        for k in range(KH):
            nc.sync.dma_start(w1blk0[:, k:k + 1, :], w1ap[:, k:k + 1, 0:FB])
            nc.sync.dma_start(hdn[:, k:k + 1, :], d_xr[:, k:k + 1, :])
            if k == 1:
                nc.sync.dma_start(b1_sb[:], d_b1r.ap())
                nc.sync.dma_start(b2_sb[:], d_b2r.ap())

        # PE warm-up: junk matmuls train the HAM clock gate to 2.4 GHz
        # while the first DMAs are in flight.
        ones_k = const.tile([P, 1], bf)
        nc.vector.memset(ones_k, 1.0)
        warm_rhs = const.tile([P, 256], bf)
        nc.vector.memset(warm_rhs, 0.0)
        ps_w = ps1.tile([1, 256], f32, tag="ps1", name="warm")
        for i in range(14):
            nc.tensor.matmul(ps_w[:], ones_k[:], warm_rhs[:],
                             start=True, stop=True)

        w2tiles = [load_w2_block(0)]
        w1tiles = [w1blk0, load_w1_block(1)]

        y_acc = ypool.tile([P, KH, C], f32, tag="y", name="y_acc")

        # ---- main loop: F-block-major, weight-stationary across chunks ----
        for fb in range(NFB):
            if fb + 2 < NFB:
                w1tiles.append(load_w1_block(fb + 2))
            if fb + 1 < NFB:
                w2tiles.append(load_w2_block(fb + 1))
            w1blk, w2blk = w1tiles[fb], w2tiles[fb]

            a_blk = apool.tile([P, MB, C], bf, tag="a", name=f"a_{fb}")
            for m in range(MB):
                psg = [ps1.tile([P, chunks[ci][1]], f32, tag="ps1",
                                name=f"ps1_{fb}_{m}_{ci}") for ci in range(NC)]
                for k in range(KH):
                    lhsT = w1blk[:, k, m * P:(m + 1) * P]
                    for ci, (off, w) in enumerate(chunks):
                        nc.tensor.matmul(psg[ci][:], lhsT,
                                         hdn[:, k, off:off + w],
                                         start=(k == 0), stop=(k == KH - 1))
                fcol = fb * MB + m
                for ci, (off, w) in enumerate(chunks):
                    nc.scalar.activation(a_blk[:, m, off:off + w], psg[ci][:],
                                         AF.Gelu_apprx_tanh,
                                         bias=b1_sb[:, fcol:fcol + 1])
            for h in range(KH):
                psg = [ps2.tile([P, chunks[ci][1]], f32, tag="ps2",
                                name=f"ps2_{fb}_{h}_{ci}") for ci in range(NC)]
                for kf in range(MB):
                    lhsT = w2blk[:, kf, h * P:(h + 1) * P]
                    for ci, (off, w) in enumerate(chunks):
                        nc.tensor.matmul(psg[ci][:], lhsT,
                                         a_blk[:, kf, off:off + w],
                                         start=(kf == 0), stop=(kf == MB - 1))
                for ci, (off, w) in enumerate(chunks):
                    sl = slice(off, off + w)
                    if fb == 0:
                        nc.scalar.activation(y_acc[:, h, sl], psg[ci][:],
                                             AF.Identity,
                                             bias=b2_sb[:, h:h + 1])
                    else:
                        nc.vector.tensor_add(y_acc[:, h, sl],
                                             y_acc[:, h, sl], psg[ci][:])
                        if fb == NFB - 1:
                            nc.sync.dma_start(
                                d_y.ap()[h * P:(h + 1) * P, sl],
                                y_acc[:, h, sl])

    nc.compile()
    _BUILD_CACHE[C] = nc
    return nc


def _prepare(x, Wg, alpha, ln_w, ln_b, fc1_w, fc1_b, fc2_w, fc2_b):
    """Host-side routing + gate + LayerNorm + per-core input construction."""
    bfnp = ml_dtypes.bfloat16
    xf = np.asarray(x, np.float32).reshape(T, H)
    Wg = np.asarray(Wg, np.float32)
    alpha = np.asarray(alpha, np.float32)
    ln_w = np.asarray(ln_w, np.float32)
    ln_b = np.asarray(ln_b, np.float32)
    fc1_w = np.asarray(fc1_w, np.float32)
    fc1_b = np.asarray(fc1_b, np.float32)
    fc2_w = np.asarray(fc2_w, np.float32)
    fc2_b = np.asarray(fc2_b, np.float32)

    logits = xf @ Wg
    order = np.argsort(-logits, axis=1, kind="stable")
    top2 = order[:, :TOP_K]
    tv = np.take_along_axis(logits, top2, 1)
    sm = np.exp(tv - tv.max(1, keepdims=True))
    sm /= sm.sum(1, keepdims=True)
    comb = np.zeros((T, E), np.float32)
    np.put_along_axis(comb, top2, sm, 1)
    comb *= alpha

    sel = np.zeros((T, E), dtype=bool)
    sel[np.arange(T)[:, None], top2] = True
    idx = [np.nonzero(sel[:, e])[0] for e in range(E)]

    mu = xf.mean(1, keepdims=True)
    var = ((xf - mu) ** 2).mean(1, keepdims=True)
    hdnb = (xf - mu) / np.sqrt(var + LN_EPS)

    maxc = max(len(i) for i in idx)
    C = max(512, 16 * ((maxc + 15) // 16))

    in_maps = []
    for e in range(E):
        n = len(idx[e])
        xg = np.zeros((C, H), np.float32)
        xg[:n] = hdnb[idx[e]] * ln_w[e] + ln_b[e]
        in_maps.append({
            "hdnT": np.ascontiguousarray(xg.T).astype(bfnp),
            "w1": fc1_w[e].astype(bfnp),
            "w2": fc2_w[e].astype(bfnp),
            "b1r": np.ascontiguousarray(fc1_b[e].reshape(MF1, P).T),
            "b2r": np.ascontiguousarray(fc2_b[e].reshape(KH, P).T),
        })
    return in_maps, idx, comb, C


def _kernel_impl(inputs, trace=False, trace_cores=None):
    from concourse import bass_utils

    in_maps, idx, comb, C = _prepare(**inputs)
    nc = _build(C)
    res = bass_utils.run_bass_kernel_spmd(
        nc, in_maps, core_ids=list(range(E)),
        trace=trace, trace_cores=trace_cores)

    out = np.zeros((T, H), np.float32)
    for e in range(E):
        yt = np.asarray(res.results[e]["ytT"], np.float32)  # [H, C]
        n = len(idx[e])
        out[idx[e]] += yt.T[:n] * comb[idx[e], e][:, None]
    return out.reshape(B, S, H), res


def kernel(**inputs):
    out, _ = _kernel_impl(inputs)
    return out


# revision 13
# speedup vs baseline: 1.0013x; 1.0013x over previous
"""Trainium2 Bass kernel for a top-2 gated MoE layer (8 experts, H=1024, F=4096).

Strategy (expert parallelism across the 8 NeuronCores):
  - Host computes routing (top-2 over gate logits), the gate softmax
    weights, and the (O(T*H), compute-trivial) LayerNorm; it gathers each
    expert's normalized tokens into a padded, transposed activation block
    hdnT [H, C] bf16 (C = padded per-expert capacity).
  - Each core runs one expert's FFN only -- the 99.9%-of-FLOPs part:
    fc1 (bf16 matmul, fp32 PSUM) -> gelu(tanh)+b1 (ACT) -> fc2 (bf16
    matmul) -> y accumulation across the 4 F-blocks in SBUF (vector adds
    on the otherwise-idle DVE).  Weight tiles stay stationary across all
    C-chunks (one LDWEIGHTS per 3 matmuls -- a fresh lhsT every matmul
    costs ~46ns of unhidden LDWEIGHTS).
  - Host scatter-adds gate-weight-scaled per-expert outputs back into the
    full [B,S,H] tensor.

Self-contained: shapes are hardcoded from the problem spec.
"""

import numpy as np
import ml_dtypes
from contextlib import ExitStack

TOP_K = 2
LN_EPS = 1e-5
B, S, H, E, F = 2, 2048, 1024, 8, 4096
T = B * S
P = 128
KH = H // P          # 8 H-tiles (fc1 contraction / fc2 output)
MF1 = F // P         # 32 fc1 output tiles
FB = 1024            # F block size for weight DMA staging
NFB = F // FB        # 4 blocks
MB = FB // P         # 8 F-tiles per block
CW = 512             # chunk width (1 PSUM bank of fp32)

_BUILD_CACHE = {}


def _chunks(C):
    out = []
    off = 0
    while C - off > CW:
        out.append((off, CW))
        off += CW
    out.append((off, C - off))
    return out


def _build(C):
    """Build + compile the single-core Bass program (SPMD across 8 cores)."""
    if C in _BUILD_CACHE:
        return _BUILD_CACHE[C]

    import concourse.bass as bass  # noqa: F401
    import concourse.tile as tile
    import concourse.mybir as mybir
    from concourse import bacc

    bf = mybir.dt.bfloat16
    f32 = mybir.dt.float32
    AF = mybir.ActivationFunctionType

    nc = bacc.Bacc("TRN2", target_bir_lowering=False, debug=False, num_devices=8)

    d_hdn = nc.dram_tensor("hdnT", [H, C], bf, kind="ExternalInput")
    d_w1 = nc.dram_tensor("w1", [H, F], bf, kind="ExternalInput")
    d_w2 = nc.dram_tensor("w2", [F, H], bf, kind="ExternalInput")
    d_b1r = nc.dram_tensor("b1r", [P, MF1], f32, kind="ExternalInput")
    d_b2r = nc.dram_tensor("b2r", [P, KH], f32, kind="ExternalInput")
    d_y = nc.dram_tensor("ytT", [H, C], f32, kind="ExternalOutput")

    chunks = _chunks(C)
    NC = len(chunks)

    with tile.TileContext(nc) as tc, ExitStack() as ctx:
        const = ctx.enter_context(tc.tile_pool(name="const", bufs=1))
        xpool = ctx.enter_context(tc.tile_pool(name="x", bufs=1))
        w1pool = ctx.enter_context(tc.tile_pool(name="w1", bufs=2))
        w2pool = ctx.enter_context(tc.tile_pool(name="w2", bufs=2))
        apool = ctx.enter_context(tc.tile_pool(name="acts", bufs=2))
        ypool = ctx.enter_context(tc.tile_pool(name="y", bufs=1))
        ps1 = ctx.enter_context(tc.tile_pool(name="ps1", bufs=4, space="PSUM"))
        ps2 = ctx.enter_context(tc.tile_pool(name="ps2", bufs=4, space="PSUM"))

        w1ap = d_w1.ap().rearrange("(k p) f -> p k f", p=P)
        w2ap = d_w2.ap().rearrange("(k p) h -> p k h", p=P)

        def load_w1_block(fb, split=1):
            w1blk = w1pool.tile([P, KH, FB], bf, tag="w1", name=f"w1_{fb}")
            step = FB // split
            for i in range(split):
                fsl = slice(fb * FB + i * step, fb * FB + (i + 1) * step)
                nc.sync.dma_start(w1blk[:, :, i * step:(i + 1) * step],
                                  w1ap[:, :, fsl])
            return w1blk

        def load_w2_block(fb):
            w2blk = w2pool.tile([P, MB, H], bf, tag="w2", name=f"w2_{fb}")
            nc.sync.dma_start(w2blk[:], w2ap[:, fb * MB:(fb + 1) * MB, :])
            return w2blk

        # ---- DMAs ordered to match fc1(fb0)'s consumption: w1 block 0 in
        # per-m-tile pieces interleaved with per-k-tile hdn rows, so the
        # first PSUM group can start ~10us in; w2 deferred. ----
        hdn = xpool.tile([P, KH, C], bf, tag="x", name="hdn")
        d_xr = d_hdn.ap().rearrange("(k p) c -> p k c", p=P)
        w1blk0 = w1pool.tile([P, KH, FB], bf, tag="w1", name="w1_0")

        # k-granular pieces keep DMA descriptors at 2KB and land in the
        # order fb0's first PSUM group consumes them.
        b1_sb = const.tile([P, MF1], f32)
        b2_sb = const.tile([P, KH], f32)
        for k in range(KH):
            nc.sync.dma_start(w1blk0[:, k:k + 1, :], w1ap[:, k:k + 1, 0:FB])
            nc.sync.dma_start(hdn[:, k:k + 1, :], d_xr[:, k:k + 1, :])
            if k == 1:
                nc.sync.dma_start(b1_sb[:], d_b1r.ap())
                nc.sync.dma_start(b2_sb[:], d_b2r.ap())

        # PE warm-up: junk matmuls train the HAM clock gate to 2.4 GHz
        # while the first DMAs are in flight.
        ones_k = const.tile([P, 1], bf)
        nc.vector.memset(ones_k, 1.0)
        warm_rhs = const.tile([P, 256], bf)
        nc.vector.memset(warm_rhs, 0.0)
        ps_w = ps1.tile([1, 256], f32, tag="ps1", name="warm")
        for i in range(14):
            nc.tensor.matmul(ps_w[:], ones_k[:], warm_rhs[:],
                             start=True, stop=True)

        w2tiles = [load_w2_block(0)]
        w1tiles = [w1blk0, load_w1_block(1)]

        y_acc = ypool.tile([P, KH, C], f32, tag="y", name="y_acc")

        # ---- main loop: F-block-major, weight-stationary across chunks ----
        for fb in range(NFB):
            if fb + 2 < NFB:
                w1tiles.append(load_w1_block(fb + 2))
            if fb + 1 < NFB:
                w2tiles.append(load_w2_block(fb + 1))
            w1blk, w2blk = w1tiles[fb], w2tiles[fb]

            a_blk = apool.tile([P, MB, C], bf, tag="a", name=f"a_{fb}")
            for m in range(MB):
                psg = [ps1.tile([P, chunks[ci][1]], f32, tag="ps1",
                                name=f"ps1_{fb}_{m}_{ci}") for ci in range(NC)]
                if fb == 0 and m < 6:
                    # dependency-free fillers: occupy fill-phase DMA stalls
                    # so the HAM clock gate trains to full width early; the
                    # real k=0 start=True resets the bank.
                    for _ in range(3):
                        nc.tensor.matmul(psg[0][0:1, 0:256], ones_k[:],
                                         warm_rhs[:], start=True, stop=True)
                for k in range(KH):
                    lhsT = w1blk[:, k, m * P:(m + 1) * P]
                    for ci, (off, w) in enumerate(chunks):
                        nc.tensor.matmul(psg[ci][:], lhsT,
                                         hdn[:, k, off:off + w],
                                         start=(k == 0), stop=(k == KH - 1))
                fcol = fb * MB + m
                for ci, (off, w) in enumerate(chunks):
                    nc.scalar.activation(a_blk[:, m, off:off + w], psg[ci][:],
                                         AF.Gelu_apprx_tanh,
                                         bias=b1_sb[:, fcol:fcol + 1])
            for h in range(KH):
                psg = [ps2.tile([P, chunks[ci][1]], f32, tag="ps2",
                                name=f"ps2_{fb}_{h}_{ci}") for ci in range(NC)]
                for kf in range(MB):
                    lhsT = w2blk[:, kf, h * P:(h + 1) * P]
                    for ci, (off, w) in enumerate(chunks):
                        nc.tensor.matmul(psg[ci][:], lhsT,
                                         a_blk[:, kf, off:off + w],
                                         start=(kf == 0), stop=(kf == MB - 1))
                for ci, (off, w) in enumerate(chunks):
                    sl = slice(off, off + w)
                    if fb == 0:
                        nc.scalar.activation(y_acc[:, h, sl], psg[ci][:],
                                             AF.Identity,
                                             bias=b2_sb[:, h:h + 1])
                    else:
                        nc.vector.tensor_add(y_acc[:, h, sl],
                                             y_acc[:, h, sl], psg[ci][:])
                        if fb == NFB - 1:
                            nc.sync.dma_start(
                                d_y.ap()[h * P:(h + 1) * P, sl],
                                y_acc[:, h, sl])

    nc.compile()
    _BUILD_CACHE[C] = nc
    return nc


def _prepare(x, Wg, alpha, ln_w, ln_b, fc1_w, fc1_b, fc2_w, fc2_b):
    """Host-side routing + gate + LayerNorm + per-core input construction."""
    bfnp = ml_dtypes.bfloat16
    xf = np.asarray(x, np.float32).reshape(T, H)
    Wg = np.asarray(Wg, np.float32)
    alpha = np.asarray(alpha, np.float32)
    ln_w = np.asarray(ln_w, np.float32)
    ln_b = np.asarray(ln_b, np.float32)
    fc1_w = np.asarray(fc1_w, np.float32)
    fc1_b = np.asarray(fc1_b, np.float32)
    fc2_w = np.asarray(fc2_w, np.float32)
    fc2_b = np.asarray(fc2_b, np.float32)

    logits = xf @ Wg
    order = np.argsort(-logits, axis=1, kind="stable")
    top2 = order[:, :TOP_K]
    tv = np.take_along_axis(logits, top2, 1)
    sm = np.exp(tv - tv.max(1, keepdims=True))
    sm /= sm.sum(1, keepdims=True)
    comb = np.zeros((T, E), np.float32)
    np.put_along_axis(comb, top2, sm, 1)
    comb *= alpha

    sel = np.zeros((T, E), dtype=bool)
    sel[np.arange(T)[:, None], top2] = True
    idx = [np.nonzero(sel[:, e])[0] for e in range(E)]

    mu = xf.mean(1, keepdims=True)
    var = ((xf - mu) ** 2).mean(1, keepdims=True)
    hdnb = (xf - mu) / np.sqrt(var + LN_EPS)

    maxc = max(len(i) for i in idx)
    C = max(512, 16 * ((maxc + 15) // 16))

    in_maps = []
    for e in range(E):
        n = len(idx[e])
        xg = np.zeros((C, H), np.float32)
        xg[:n] = hdnb[idx[e]] * ln_w[e] + ln_b[e]
        in_maps.append({
            "hdnT": np.ascontiguousarray(xg.T).astype(bfnp),
            "w1": fc1_w[e].astype(bfnp),
            "w2": fc2_w[e].astype(bfnp),
            "b1r": np.ascontiguousarray(fc1_b[e].reshape(MF1, P).T),
            "b2r": np.ascontiguousarray(fc2_b[e].reshape(KH, P).T),
        })
    return in_maps, idx, comb, C


def _kernel_impl(inputs, trace=False, trace_cores=None):
    from concourse import bass_utils

    in_maps, idx, comb, C = _prepare(**inputs)
    nc = _build(C)
    res = bass_utils.run_bass_kernel_spmd(
        nc, in_maps, core_ids=list(range(E)),
        trace=trace, trace_cores=trace_cores)

    out = np.zeros((T, H), np.float32)
    for e in range(E):
        yt = np.asarray(res.results[e]["ytT"], np.float32)  # [H, C]
        n = len(idx[e])
        out[idx[e]] += yt.T[:n] * comb[idx[e], e][:, None]
    return out.reshape(B, S, H), res


def kernel(**inputs):
    out, _ = _kernel_impl(inputs)
    return out
